# revision 2
# baseline (speedup 1.0000x reference)
"""DA-RNN decoder (input-attention LSTM) Bass kernel for Trainium2, 8 cores.

Algorithm (per time step t, per sample b):
    hproj[r]   = sum_k W_h[r,k] * hs[k]          hs = [h; c], r in [S]
    z[r,d]     = tanh(hproj[r] + x_proj[d,r])    x_proj precomputed from x, W_x
    E[d]       = sum_r v[r] * z[r,d]
    alpha      = softmax_d(E)                    (no max-subtraction; |E| <~ 9)
    inp        = x[b,t,:] * alpha
    gates      = inp @ W_ih.T + h @ W_hh.T (+ b)
    i,f,g,o    = split(gates);  LSTM cell update; out[t] = h_new

Sharding: data-parallel over batch B=128 -> 16 samples on each of 8 cores;
weights replicated. Each core runs an identical NEFF on its x-slice.

Numeric tricks baked into the (numpy-preprocessed) weights:
  * sigmoid(x) = 0.5*tanh(x/2)+0.5  -> i,f,o rows of W_ih/W_hh/b scaled 0.5
    so ONE tanh activation covers all four gates.
  * LSTM states are stored doubled (h2=2h, c2=2c); the 0.5 compensations are
    folded into W_h (x0.5) and W_hh (x0.5); output h = h2*0.5 on the way out.

Everything LSTM-side lives TRANSPOSED ([128 partitions = gate/state dim,
free = samples]) so ACT/DVE ops are free-dim-light:
  x_sd     [S=128p, bc, D]     for x_proj matmuls (rhs)
  x_bd     [bc=16p, S, D]      xt slices (transposed per-step on PE)
  x_projT  [r=128p, bc, D]     z argument, precomputed once on PE
  h2T/c2T  [128p, 2, bc]       doubled transposed states (f32r)
  gatesT   [128p, 8, bc] psum  gate blocks: i=0:2, f=2:4, g=4:6, o=6:8
  E_T      [d=128p, bc] via per-sample matmuls (lhsT = z_b bf16, rhs = v)
"""

import numpy as np

import concourse.bass as bass
import concourse.bacc as bacc
import concourse.tile as tile
from concourse import mybir
from concourse.bass_utils import run_bass_kernel_spmd
from concourse.masks import make_identity

B, S, D, H = 128, 128, 128, 256
NCORE = 8
BC = B // NCORE          # 16 samples per core
H2 = 2 * H               # 512
G4 = 4 * H               # 1024
P = 128

F32 = mybir.dt.float32
F32R = mybir.dt.float32r
BF16 = mybir.dt.bfloat16
ALU = mybir.AluOpType
AF = mybir.ActivationFunctionType

NCHUNK = 2               # z-phase sample chunks
CB = BC // NCHUNK        # samples per chunk
POOL_CHUNK = NCHUNK - 1  # which z-add chunk goes to GPSIMD (-1: none)


def build_nc(has_bias: bool):
    nc = bacc.Bacc(target_bir_lowering=False, debug=False)

    x_d = nc.dram_tensor("x", [BC, S, D], F32, kind="ExternalInput")
    whT_d = nc.dram_tensor("whT", [H2, S], F32, kind="ExternalInput")
    wxT_d = nc.dram_tensor("wxT", [S, S], F32, kind="ExternalInput")
    # transposed-gate weight blocks: [dpart, gblock, grow] / [kpart, kb, gblock, grow]
    wih2_d = nc.dram_tensor("wih2", [D, 8, P], F32, kind="ExternalInput")
    whh2_d = nc.dram_tensor("whh2", [P, 2, 8, P], F32, kind="ExternalInput")
    v_d = nc.dram_tensor("v", [S, 1], BF16, kind="ExternalInput")
    if has_bias:
        bias_d = nc.dram_tensor("biasT", [P, 8], F32, kind="ExternalInput")
    out_d = nc.dram_tensor("out", [S, BC, H], F32, kind="ExternalOutput")

    with tile.TileContext(nc) as tc:
        with tc.tile_pool(name="singles", bufs=1) as singles:
            ident = singles.tile([P, P], F32)
            make_identity(nc, ident)
            id16 = ident[:BC, :BC]

            whT_sb = singles.tile([P, 4, S], F32)
            nc.sync.dma_start(whT_sb, whT_d.ap().rearrange("(k p) m -> p k m", p=P))
            wxT_sb = singles.tile([P, S], F32)
            nc.sync.dma_start(wxT_sb, wxT_d[:])
            wih2_sb = singles.tile([P, 8, P], F32)
            nc.sync.dma_start(wih2_sb, wih2_d[:])
            whh2_sb = singles.tile([P, 2, 8, P], F32)
            nc.sync.dma_start(whh2_sb, whh2_d[:])
            v_sb = singles.tile([P, 1], BF16)
            nc.sync.dma_start(v_sb, v_d[:])
            ones_col = singles.tile([P, 1], F32)
            nc.vector.memset(ones_col, 1.0)
            if has_bias:
                biasT_sb = singles.tile([P, 8], F32)
                nc.sync.dma_start(biasT_sb, bias_d[:])

            x_sd = singles.tile([P, BC, D], F32)
            nc.sync.dma_start(x_sd, x_d.ap().rearrange("b s d -> s b d"))
            x_bd = singles.tile([BC, S, D], F32)
            nc.sync.dma_start(x_bd, x_d[:])

            x_projT = singles.tile([P, BC, D], F32)

            # persistent transposed states (doubled): [128, 2(block), bc]
            h2T = singles.tile([P, 2, BC], F32)
            c2T = singles.tile([P, 2, BC], F32)
            nc.vector.memset(h2T, 0.0)
            nc.vector.memset(c2T, 0.0)

            # ---- x_projT precompute: x_projT[r, b, d] = sum_s W_x[r,s] x[b,s,d]
            with tc.tile_pool(name="pre_ps", bufs=4, space="PSUM") as pre_ps:
                for b in range(BC):
                    xp_ps = pre_ps.tile([P, D], F32, tag="xp")
                    nc.tensor.matmul(
                        xp_ps, wxT_sb, x_sd[:, b, :], start=True, stop=True
                    )
                    nc.vector.tensor_copy(x_projT[:, b, :], xp_ps)

            with (
                tc.tile_pool(name="step_sb", bufs=3) as sb,
                tc.tile_pool(name="state_sb", bufs=2) as st,
                tc.tile_pool(name="ps_a", bufs=1, space="PSUM") as ps_a,
                tc.tile_pool(name="ps_g", bufs=1, space="PSUM") as ps_g,
                tc.tile_pool(name="ps_g2", bufs=1, space="PSUM") as ps_g2,
                tc.tile_pool(name="ps_e", bufs=1, space="PSUM") as ps_e,
                tc.tile_pool(name="ps_t", bufs=1, space="PSUM") as ps_t,
                tc.tile_pool(name="ps_o", bufs=2, space="PSUM") as ps_o,
            ):
                def emit_xt_transpose(t):
                    xt_ps = ps_t.tile([P, BC], F32, tag="xt_ps")
                    nc.tensor.transpose(xt_ps, x_bd[:, t, :], id16)
                    xtT = st.tile([P, BC], F32, tag="xtT", name=f"xtT_{t}")
                    nc.vector.tensor_copy(xtT, xt_ps)
                    return xtT

                xtT_sb = emit_xt_transpose(0)

                for t in range(S):
                    # ---- hproj (r x b), 4 accumulating f32r matmuls
                    hproj_ps = ps_a.tile([P, BC], F32, tag="hproj", name=f"hproj_{t}")
                    rhs_blocks = (
                        h2T[:, 0, :], h2T[:, 1, :], c2T[:, 0, :], c2T[:, 1, :],
                    )
                    for k in range(4):
                        nc.tensor.matmul(
                            hproj_ps,
                            whT_sb[:, k, :],
                            rhs_blocks[k],
                            start=(k == 0),
                            stop=(k == 3),
                            skip_group_check=True,
                        )
                    hp_sb = sb.tile([P, BC], F32, tag="hp", name=f"hp_{t}")
                    nc.vector.tensor_copy(hp_sb, hproj_ps)

                    # ---- gates hh-part (early): gatesT psum [128, 8, BC]
                    ghh_ps = ps_g.tile([P, 8, BC], F32, tag="ghh", name=f"ghh_{t}")
                    for g in range(8):
                        for kb in range(2):
                            nc.tensor.matmul(
                                ghh_ps[:, g, :],
                                whh2_sb[:, kb, g, :],
                                h2T[:, kb, :],
                                start=(kb == 0),
                                stop=(kb == 1),
                                skip_group_check=True,
                            )
                    ghh_sb = sb.tile([P, 8, BC], F32, tag="ghh_sb", name=f"ghhs_{t}")
                    nc.vector.tensor_copy(ghh_sb, ghh_ps)

                    # ---- z phase: zarg = x_projT + hproj (broadcast over d)
                    ET_ps = ps_e.tile([P, BC], F32, tag="ET", name=f"ET_{t}")
                    for c in range(NCHUNK):
                        bs = c * CB
                        zarg = sb.tile([P, CB, D], F32, tag="zarg", name=f"zarg_{t}_{c}")
                        eng = nc.gpsimd if c == POOL_CHUNK else nc.vector
                        eng.tensor_tensor(
                            zarg,
                            x_projT[:, bs:bs + CB, :],
                            hp_sb[:, bs:bs + CB, None].to_broadcast([P, CB, D]),
                            ALU.add,
                        )
                        z_bf = sb.tile([P, CB, D], BF16, tag="zbf", name=f"zbf_{t}_{c}")
                        nc.scalar.activation(z_bf, zarg, AF.Tanh)
                        for j in range(CB):
                            nc.tensor.matmul(
                                ET_ps[:, bs + j:bs + j + 1],
                                z_bf[:, j, :],
                                v_sb,
                                start=True,
                                stop=True,
                                skip_group_check=True,
                            )

                    # ---- softmax over d (partition dim of ET)
                    expET = sb.tile([P, BC], F32, tag="expET", name=f"expET_{t}")
                    nc.scalar.activation(expET, ET_ps, AF.Exp)
                    ssum_ps = ps_t.tile([1, BC], F32, tag="ssum", name=f"ssum_{t}")
                    nc.tensor.matmul(ssum_ps, ones_col, expET, start=True, stop=True)
                    recip_sb = sb.tile([1, BC], F32, tag="recip", name=f"recip_{t}")
                    nc.vector.reciprocal(recip_sb, ssum_ps)
                    rb_sb = sb.tile([P, BC], F32, tag="rb", name=f"rb_{t}")
                    nc.gpsimd.partition_broadcast(rb_sb, recip_sb)

                    # ---- inp_T = xt_T * expE_T * rb   (f32r for the ih matmuls)
                    tmp_sb = sb.tile([P, BC], F32, tag="tmp", name=f"tmp_{t}")
                    nc.vector.tensor_tensor(tmp_sb, xtT_sb, expET, ALU.mult)
                    inpT_sb = sb.tile([P, BC], F32, tag="inpT", name=f"inpT_{t}")
                    nc.vector.tensor_tensor(inpT_sb, tmp_sb, rb_sb, ALU.mult)

                    # ---- gates ih-part (completes the psum accumulation)
                    gih_ps = ps_g2.tile([P, 8, BC], F32, tag="gih", name=f"gih_{t}")
                    for g in range(8):
                        nc.tensor.matmul(
                            gih_ps[:, g, :],
                            wih2_sb[:, g, :],
                            inpT_sb,
                            start=True,
                            stop=True,
                            skip_group_check=True,
                        )
                    gsum_sb = sb.tile([P, 8, BC], F32, tag="gsum", name=f"gsum_{t}")
                    nc.vector.tensor_tensor(gsum_sb, gih_ps, ghh_sb, ALU.add)

                    # ---- gate nonlinearity: one tanh covers i,f,g,o
                    if has_bias:
                        nc.vector.tensor_tensor(
                            gsum_sb, gsum_sb,
                            biasT_sb[:, :, None].to_broadcast([P, 8, BC]),
                            ALU.add,
                        )
                    T_sb = sb.tile([P, 8, BC], F32, tag="T", name=f"T_{t}")
                    nc.scalar.activation(T_sb, gsum_sb, AF.Tanh)

                    # ---- LSTM cell, transposed ([128, 2, BC] slices)
                    Ti = T_sb[:, 0:2, :]
                    Tf = T_sb[:, 2:4, :]
                    Tg = T_sb[:, 4:6, :]
                    To = T_sb[:, 6:8, :]
                    sf_sb = sb.tile([P, 2, BC], F32, tag="sf", name=f"sf_{t}")
                    nc.vector.tensor_scalar(
                        sf_sb, Tf, 0.5, 0.5, op0=ALU.mult, op1=ALU.add
                    )
                    A_sb = sb.tile([P, 2, BC], F32, tag="A", name=f"A_{t}")
                    nc.vector.tensor_tensor(A_sb, sf_sb, c2T, ALU.mult)
                    B_sb = sb.tile([P, 2, BC], F32, tag="Bt", name=f"B_{t}")
                    nc.vector.scalar_tensor_tensor(
                        B_sb, Ti, 1.0, Tg, op0=ALU.add, op1=ALU.mult
                    )
                    nc.vector.tensor_tensor(c2T, A_sb, B_sb, ALU.add)
                    tc_sb = sb.tile([P, 2, BC], F32, tag="tc", name=f"tc_{t}")
                    nc.scalar.activation(tc_sb, c2T, AF.Tanh, scale=0.5)
                    nc.vector.scalar_tensor_tensor(
                        h2T, To, 1.0, tc_sb, op0=ALU.add, op1=ALU.mult
                    )

                    # ---- output: transpose h2T -> [32, 128], halve, DMA out
                    ho_ps = ps_o.tile([2 * BC, P], F32, tag="ho", name=f"ho_{t}")
                    nc.tensor.transpose(
                        ho_ps, h2T.rearrange("p a b -> p (a b)"),
                        ident[:, :],
                    )
                    hout_sb = sb.tile([2 * BC, P], F32, tag="hout", name=f"hout_{t}")
                    nc.vector.tensor_scalar_mul(hout_sb, ho_ps, 0.5)
                    nc.sync.dma_start(
                        out_d[t].rearrange("b (hb hi) -> hb b hi", hb=2),
                        hout_sb,
                    )

                    if t + 1 < S:
                        xtT_sb = emit_xt_transpose(t + 1)

    nc.finalize()
    return nc


_NC_CACHE = {}


def _preprocess(W_WU, W_v, W_ih, W_hh, b_ih, b_hh):
    import ml_dtypes

    gs = np.ones(G4, dtype=np.float32)
    gs[0:512] = 0.5          # i, f
    gs[768:1024] = 0.5       # o
    W_h = W_WU[:, :H2]
    W_x = W_WU[:, H2:]
    whT = np.ascontiguousarray(W_h.T) * np.float32(0.5)      # doubled-state fold
    wxT = np.ascontiguousarray(W_x.T)
    wihT = (W_ih * gs[:, None]).T                             # [D, G4]
    wih2 = np.ascontiguousarray(wihT.reshape(D, 8, P))
    whhT = (W_hh * gs[:, None]).T * np.float32(0.5)           # [H, G4]
    whh2 = np.ascontiguousarray(
        whhT.reshape(2, P, 8, P).transpose(1, 0, 2, 3)
    )
    v_bf = np.ascontiguousarray(W_v.reshape(S, 1)).astype(ml_dtypes.bfloat16)
    bias = (b_ih + b_hh) * gs
    biasT = np.ascontiguousarray(bias.reshape(8, P).T)        # [P, 8]
    return whT, wxT, wih2, whh2, v_bf, biasT


def kernel(x, W_WU, W_v, W_ih, W_hh, b_ih, b_hh):
    x = np.ascontiguousarray(np.asarray(x, dtype=np.float32))
    whT, wxT, wih2, whh2, v_bf, biasT = _preprocess(
        np.asarray(W_WU, dtype=np.float32),
        np.asarray(W_v, dtype=np.float32),
        np.asarray(W_ih, dtype=np.float32),
        np.asarray(W_hh, dtype=np.float32),
        np.asarray(b_ih, dtype=np.float32),
        np.asarray(b_hh, dtype=np.float32),
    )
    has_bias = bool(np.any(biasT))

    if has_bias not in _NC_CACHE:
        _NC_CACHE[has_bias] = build_nc(has_bias)
    nc = _NC_CACHE[has_bias]

    in_maps = []
    for c in range(NCORE):
        m = {
            "x": np.ascontiguousarray(x[c * BC:(c + 1) * BC]),
            "whT": whT,
            "wxT": wxT,
            "wih2": wih2,
            "whh2": whh2,
            "v": v_bf,
        }
        if has_bias:
            m["biasT"] = biasT
        in_maps.append(m)

    res = run_bass_kernel_spmd(nc, in_maps, core_ids=list(range(NCORE)))
    outs = np.concatenate([res.results[c]["out"] for c in range(NCORE)], axis=1)
    return outs.astype(np.float32)


def _trace_in_maps(inputs):
    """Build the per-core in_maps for an already-compiled kernel (test harness)."""
    x = np.ascontiguousarray(np.asarray(inputs["x"], dtype=np.float32))
    whT, wxT, wih2, whh2, v_bf, biasT = _preprocess(
        np.asarray(inputs["W_WU"], dtype=np.float32),
        np.asarray(inputs["W_v"], dtype=np.float32),
        np.asarray(inputs["W_ih"], dtype=np.float32),
        np.asarray(inputs["W_hh"], dtype=np.float32),
        np.asarray(inputs["b_ih"], dtype=np.float32),
        np.asarray(inputs["b_hh"], dtype=np.float32),
    )
    has_bias = bool(np.any(biasT))
    in_maps = []
    for c in range(NCORE):
        m = {
            "x": np.ascontiguousarray(x[c * BC:(c + 1) * BC]),
            "whT": whT,
            "wxT": wxT,
            "wih2": wih2,
            "whh2": whh2,
            "v": v_bf,
        }
        if has_bias:
            m["biasT"] = biasT
        in_maps.append(m)
    return in_maps


if __name__ == "__main__":
    rng = np.random.default_rng(0)
    xs = {
        "x": rng.standard_normal((B, S, D), dtype=np.float32),
        "W_WU": (rng.standard_normal((S, H2 + S), dtype=np.float32)
                 / np.sqrt(H2 + S)),
        "W_v": rng.standard_normal((1, S), dtype=np.float32) / np.sqrt(S),
        "W_ih": rng.standard_normal((G4, D), dtype=np.float32) / np.sqrt(D),
        "W_hh": rng.standard_normal((G4, H), dtype=np.float32) / np.sqrt(H),
        "b_ih": np.zeros(G4, dtype=np.float32),
        "b_hh": np.zeros(G4, dtype=np.float32),
    }
    out = kernel(**xs)
    print("kernel out", out.shape, out.dtype, np.abs(out).mean())

